# revision 10
# baseline (speedup 1.0000x reference)
"""Causal self-attention (B=4, T=2048, C=1024, H=16) on 8 trn2 NeuronCores.

Sharding: core = (batch b, head-group g) with b in 0..3, g in 0..1.
Each core computes attention for batch b over 8 local heads (g*8 .. g*8+8)
plus the partial output projection for those heads' features.  The host
sums the two partial projections per batch and adds b_proj.

Per-core kernel (all matmuls bf16 inputs, fp32 PSUM accumulation):
  xT   : PE-transpose of x (fp32) -> bf16 [C, T] in SBUF
  qkvT : w_attn-stationary matmuls -> qT/kT per head-pair [128, T]
  v    : x-stationary matmuls -> v natural [T, 8, 64] + fused ones column
  attn : S^T blocks = kT.T @ qT (head-pair row-tiled, K=64), ACT exp
         (scale=1/8), triangular mask on diagonal 128x128 sub-blocks only,
         PV matmul with v_aug [128, 65] stationary -> yT accum + softmax
         sums in row 64 (the ones column), block-causal skipping.
  norm : sums -> partition_broadcast -> reciprocal -> yT bf16
  proj : yT-stationary matmuls vs w_proj -> out partial [T, C] fp32
"""

import numpy as np

P = 128
B, T, C = 4, 2048, 1024
H, D = 16, 64
HL = 8            # local heads per core
CL = HL * D       # 512 local feature cols per group
NT = T // P       # 16 t-tiles
NTC = T // 512    # 4 t-chunks
NCB = C // P      # 8 c-tiles

_cache = {}


def _build_nc():
    import concourse.tile as tile
    from concourse import bacc, mybir
    from concourse.masks import make_identity, make_upper_triangular

    f32 = mybir.dt.float32
    bf16 = mybir.dt.bfloat16
    Alu = mybir.AluOpType

    nc = bacc.Bacc(
        "TRN2", target_bir_lowering=False, debug=False, enable_asserts=False
    )
    x_d = nc.dram_tensor("x", [T, C], f32, kind="ExternalInput").ap()
    wa_d = nc.dram_tensor("w_attn", [C, 3 * CL], f32, kind="ExternalInput").ap()
    ba_d = nc.dram_tensor("b_attn", [3 * CL], f32, kind="ExternalInput").ap()
    wp_d = nc.dram_tensor("w_proj", [CL, C], f32, kind="ExternalInput").ap()
    out_d = nc.dram_tensor("out", [T, C], f32, kind="ExternalOutput").ap()

    with tile.TileContext(nc) as tc:
        with (
            tc.tile_pool(name="singles", bufs=1) as singles,
            tc.tile_pool(name="xstage", bufs=4) as xstage,
            tc.tile_pool(name="wstage", bufs=2) as wstage,
            tc.tile_pool(name="xt", bufs=1) as xtp,
            tc.tile_pool(name="qk", bufs=1) as qkp,
            tc.tile_pool(name="vpool", bufs=1) as vp,
            tc.tile_pool(name="yt", bufs=1) as ytp,
            tc.tile_pool(name="pt", bufs=4) as ptp,
            tc.tile_pool(name="small", bufs=4) as smallp,
            tc.tile_pool(name="ostage", bufs=3) as ostage,
            tc.tile_pool(name="ps_misc", bufs=2, space="PSUM") as ps_misc,
            tc.tile_pool(name="ps_s", bufs=2, space="PSUM") as ps_s,
            tc.tile_pool(name="ps_y", bufs=2, space="PSUM") as ps_y,
        ):
            # ---- constants ----
            ident = singles.tile([P, P], f32, tag="ident")
            make_identity(nc, ident)
            # tri[k, q] = 1 if q >= k else 0 (multiplicative causal mask for
            # the diagonal 128x128 sub-block of S^T)
            tri = singles.tile([P, P], bf16, tag="tri")
            make_upper_triangular(nc, tri, val=1.0, diag=True)

            b_row = singles.tile([1, 3 * CL], f32, tag="b_row")
            nc.sync.dma_start(out=b_row, in_=ba_d.rearrange("(a c) -> a c", a=1))
            # q/k bias arranged per col-tile: b_qk[p, ct] = b_attn[ct*128 + p]
            b_qk = singles.tile([P, 8], f32, tag="b_qk")
            nc.sync.dma_start(
                out=b_qk, in_=ba_d[0 : 2 * CL].rearrange("(ct p) -> p ct", p=P)
            )
            # v bias broadcast across partitions (DRAM step-0 DMA)
            b_vb = singles.tile([P, CL], f32, tag="b_vb")
            nc.gpsimd.dma_start(
                out=b_vb,
                in_=ba_d[2 * CL : 3 * CL].rearrange("(a c) -> a c", a=1).to_broadcast([P, CL]),
            )

            # ---- weights (cast fp32 -> bf16) ----
            wa = singles.tile([P, NCB, 3 * CL], bf16, tag="wa")
            for cb in range(NCB):
                st = wstage.tile([P, 3 * CL], f32, tag="wstage")
                nc.sync.dma_start(out=st, in_=wa_d[cb * P : (cb + 1) * P, :])
                nc.vector.tensor_copy(out=wa[:, cb, :], in_=st)
            wp = singles.tile([P, 4, C], bf16, tag="wp")
            for cb in range(4):
                st = wstage.tile([P, 3 * CL], f32, tag="wstage")
                nc.sync.dma_start(out=st[:, :C], in_=wp_d[cb * P : (cb + 1) * P, :])
                nc.vector.tensor_copy(out=wp[:, cb, :], in_=st[:, :C])

            # ---- x load + PE transpose -> xT bf16 [C, T] ----
            xT = [xtp.tile([P, T], bf16, tag=f"xT{cb}", name=f"xT{cb}") for cb in range(NCB)]
            for g in range(4):
                xs = []
                for j in range(4):
                    tt = g * 4 + j
                    xs_t = xstage.tile([P, C], f32, tag="xs")
                    nc.sync.dma_start(out=xs_t, in_=x_d[tt * P : (tt + 1) * P, :])
                    xs.append(xs_t)
                for cb in range(NCB):
                    ps = ps_misc.tile([P, 512], f32, tag="ps_misc")
                    for j in range(4):
                        nc.tensor.transpose(
                            ps[:, j * P : (j + 1) * P],
                            xs[j][:, cb * P : (cb + 1) * P],
                            ident,
                        )
                    nc.vector.tensor_copy(
                        out=xT[cb][:, g * 512 : (g + 1) * 512], in_=ps
                    )

            # ---- qT / kT per head pair: qk[pr] = qT pair, qk[4+pr] = kT pair
            qk = [qkp.tile([P, T], bf16, tag=f"qk{i}", name=f"qk{i}") for i in range(8)]
            for cbc in range(8):
                for tch in range(NTC):
                    ps = ps_misc.tile([P, 512], f32, tag="ps_misc")
                    for cb in range(NCB):
                        nc.tensor.matmul(
                            ps,
                            wa[:, cb, cbc * P : (cbc + 1) * P],
                            xT[cb][:, tch * 512 : (tch + 1) * 512],
                            start=(cb == 0),
                            stop=(cb == NCB - 1),
                        )
                    nc.vector.tensor_scalar(
                        out=qk[cbc][:, tch * 512 : (tch + 1) * 512],
                        in0=ps,
                        scalar1=b_qk[:, cbc : cbc + 1],
                        scalar2=None,
                        op0=Alu.add,
                    )

            # ---- v natural [t, h, d] + ones column ----
            vt = [vp.tile([P, HL, 2 * D], bf16, tag=f"v{t_}", name=f"v{t_}") for t_ in range(NT)]
            for t_ in range(NT):
                ps = ps_misc.tile([P, 512], f32, tag="ps_misc")
                for cb in range(NCB):
                    nc.tensor.matmul(
                        ps,
                        xT[cb][:, t_ * P : (t_ + 1) * P],
                        wa[:, cb, 2 * CL : 3 * CL],
                        start=(cb == 0),
                        stop=(cb == NCB - 1),
                    )
                nc.vector.tensor_tensor(
                    out=vt[t_][:, :, 0:D],
                    in0=ps.rearrange("p (h d) -> p h d", h=HL),
                    in1=b_vb.rearrange("p (h d) -> p h d", h=HL),
                    op=Alu.add,
                )
                nc.vector.memset(vt[t_][:, :, D : 2 * D], 1.0)

            # ---- attention + projection, chunk-outer ----
            yT = [ytp.tile([P, T], bf16, tag=f"yT{pr}", name=f"yT{pr}") for pr in range(4)]
            for ch in range(NTC):
                Qs = ch * 512
                KB = ch * 4 + 4  # causal: k blocks 0 .. KB-1
                for pr in range(4):
                    qTp, kTp = qk[pr], qk[4 + pr]
                    yps = [ps_y.tile([P, 512], f32, tag="ps_y", name="yps") for _ in range(2)]
                    for kb in range(KB):
                        o = max(0, kb * P - Qs)
                        sps = ps_s.tile([P, 2, 512], f32, tag="ps_s")
                        for j in range(2):
                            nc.tensor.matmul(
                                sps[:, j, o:512],
                                kTp[j * D : (j + 1) * D, kb * P : (kb + 1) * P],
                                qTp[j * D : (j + 1) * D, Qs + o : Qs + 512],
                                start=True,
                                stop=True,
                            )
                        pt = ptp.tile([P, 2, 512], bf16, tag="pt")
                        nc.scalar.activation(
                            out=pt[:, :, o:512],
                            in_=sps[:, :, o:512],
                            func=mybir.ActivationFunctionType.Exp,
                            scale=0.125,
                        )
                        if kb * P >= Qs:  # diagonal block: triangular mask
                            for j in range(2):
                                nc.vector.tensor_tensor(
                                    out=pt[:, j, o : o + P],
                                    in0=pt[:, j, o : o + P],
                                    in1=tri,
                                    op=Alu.mult,
                                )
                        for j in range(2):
                            nc.tensor.matmul(
                                yps[j][:, o:512],
                                vt[kb][:, pr * 2 + j, :],
                                pt[:, j, o:512],
                                start=(kb == 0),
                                stop=(kb == KB - 1),
                                skip_group_check=True,
                            )
                    for j in range(2):
                        rc = smallp.tile([D, 512], f32, tag="rc", name="rc")
                        nc.vector.reciprocal(rc, yps[j][D : 2 * D, :])
                        nc.vector.tensor_tensor(
                            out=yT[pr][j * D : (j + 1) * D, Qs : Qs + 512],
                            in0=yps[j][0:D, :],
                            in1=rc,
                            op=Alu.mult,
                        )
                # output projection for this chunk's t-tiles
                for t_ in range(ch * 4, ch * 4 + 4):
                    for nch in range(2):
                        pps = ps_misc.tile([P, 512], f32, tag="ps_misc")
                        for cb4 in range(4):
                            nc.tensor.matmul(
                                pps,
                                yT[cb4][:, t_ * P : (t_ + 1) * P],
                                wp[:, cb4, nch * 512 : (nch + 1) * 512],
                                start=(cb4 == 0),
                                stop=(cb4 == 3),
                            )
                        osb = ostage.tile([P, 512], f32, tag="osb", name="osb")
                        nc.vector.tensor_copy(out=osb, in_=pps)
                        nc.sync.dma_start(
                            out=out_d[t_ * P : (t_ + 1) * P, nch * 512 : (nch + 1) * 512],
                            in_=osb,
                        )
    nc.compile()
    return nc


def get_nc():
    if "nc" not in _cache:
        _cache["nc"] = _build_nc()
    return _cache["nc"]


def make_in_maps(x, w_attn, b_attn, w_proj):
    x = np.ascontiguousarray(np.asarray(x, dtype=np.float32))
    w_attn = np.asarray(w_attn, dtype=np.float32)
    b_attn = np.asarray(b_attn, dtype=np.float32)
    w_proj = np.asarray(w_proj, dtype=np.float32)
    in_maps = []
    for core in range(8):
        b, g = core // 2, core % 2
        cols = slice(g * CL, (g + 1) * CL)
        wa_l = np.concatenate(
            [w_attn[:, 0 * C :][:, cols], w_attn[:, 1 * C :][:, cols],
             w_attn[:, 2 * C :][:, cols]],
            axis=1,
        )
        ba_l = np.concatenate(
            [b_attn[0 * C :][cols], b_attn[1 * C :][cols], b_attn[2 * C :][cols]]
        )
        wp_l = w_proj[g * CL : (g + 1) * CL, :]
        in_maps.append(
            {
                "x": np.ascontiguousarray(x[b]),
                "w_attn": np.ascontiguousarray(wa_l),
                "b_attn": np.ascontiguousarray(ba_l),
                "w_proj": np.ascontiguousarray(wp_l),
            }
        )
    return in_maps


def combine_outputs(outs, b_proj):
    b_proj = np.asarray(b_proj, dtype=np.float32)
    return np.stack(
        [outs[2 * b] + outs[2 * b + 1] + b_proj[None, :] for b in range(B)]
    ).astype(np.float32)


def kernel(**inputs):
    from concourse.bass_utils import run_bass_kernel_spmd

    nc = get_nc()
    in_maps = make_in_maps(
        inputs["x"], inputs["w_attn"], inputs["b_attn"], inputs["w_proj"]
    )
    res = run_bass_kernel_spmd(nc, in_maps, core_ids=list(range(8)))
    globals()["_last_results"] = res
    outs = [r["out"] for r in res.results]
    return combine_outputs(outs, inputs["b_proj"])
